# revision 2
# baseline (speedup 1.0000x reference)
"""MultiHeadAttention (B=2, S=2048, D=1024, H=16) on 8 NeuronCores.

Sharding: data-parallel over batch (2) x tensor-parallel over heads (4 groups
of 4 heads). Core c handles batch c//4, heads (c%4)*4 .. +4. Host sums the 4
partial outputs per batch (the "all-reduce" of row-parallel Wo).

Final: natural-layout PV + software-pipelined qb-major schedule.

Walk order is qb-major (heads inner): the first 32-slot window consumes only
seq-quarter 0 of x/K/V while quarters 1-3 stream in behind it, so the serial
~39us input DMA hides almost entirely under compute. Projections for quarter
j+1 and Wo for completed q-blocks ride as fillers inside window j. Output
DMAs issue from the otherwise-idle GpSimd queue.

Data layout (vs the duplicated-QT baseline), driven by the TimelineSim cost
model where matmul cost = out-free-size x pe_cycle and activation cost =
free-size x cycle_t:
  - P.V runs in natural layout: out[q=128, d=65] += P[k,q]^T V[k,d] -- the
    65-row output uses the full 128-partition q dim, halving PV tensor time
    relative to the baseline's [65, 512] orientation,
  - softmax denominators ride along as V's ones-column (column 64 of the PV
    accumulator); normalization is a per-partition reciprocal + tensor_scalar
    multiply (no PE broadcast matmul, no cross-partition work),
  - Wo consumes x^T[d, q], produced by cheap PE transpose matmuls of the
    normalized [q, d] tiles,
  - Q^T/K^T are NOT duplicated across row-group halves: head h lives on
    partitions (h%2)*64..+64 straight out of its projection eviction.

Issue order: the exp stream on ScalarE is the pacer (~1.04us per 128x1024
tile, 128 tiles). Per pr-slot the PE stream is [st_pair(pr); PV(pr-1);
filler], so the score matmuls for the NEXT exp are always ahead of the PV
batch that blocks on the CURRENT exp, and projection/Wo matmuls are injected
in sub-us chunks to fill PE's per-slot slack without starving ScalarE.

PSUM (8 banks): "ps" 3x[128,1024]f32 (6 banks) for the score tiles and the
pre-attention projections; "pp" 1x 1 bank for interleaved filler psums
([128,512]f32 projections / wo, [128,2,2,128]bf16 transposes) -- fillers hold
their psum across slots, so they must NOT share the st rotation; "px" 1 bank
for the PV accumulator.
"""

import numpy as np
import ml_dtypes

B, S, D, H = 2, 2048, 1024, 16
HD = 64
NCORES = 8
GROUPS = 4            # head groups (tensor-parallel degree per batch)
HPC = H // GROUPS     # 4 heads per core
DSL = HPC * HD        # 256: per-core slice of D
KT = D // 128         # 8 contraction tiles for projections
SC = S // 128         # 16 sequence chunks
QB = 512              # q-block for attention phase
NQB = S // QB         # 4

_cached_nc = None
TRACE = False
TRACE_KW = {}
_last_result = None

# scheduling tunables
STEXP_BUFS = 14
NORM_BUFS = 2
OUTST_BUFS = 2
XIN_BUFS = 3
ST_BUFS = 3           # [128,1024] f32 score tiles (2 banks each)
PP_BUFS = 1           # 1-bank filler psums
PX_BUFS = 1           # [128,4,65] f32 PV accumulators (1 bank)


def _split_excess_waits(nc, mybir, max_waits=1):
    # walrus (core_v3) rejects instructions carrying more sync waits than the
    # ISA struct holds; hoist extras onto preceding same-engine NoOps.
    for fn in nc.m.functions:
        for bb in fn.blocks:
            insts = bb.instructions
            new_list = []
            changed = False
            for inst in insts:
                si = inst.sync_info
                waits = list(si.on_wait) if si and si.on_wait else []
                lim = 2 if isinstance(inst, mybir.InstEventSemaphore) else max_waits
                if len(waits) > lim:
                    for j, w in enumerate(waits[lim:]):
                        new_list.append(
                            mybir.InstNoOp(
                                name=f"{inst.name}-wsplit{j}",
                                sync_info=mybir.SyncInfo(on_wait=[w], on_update=[]),
                                engine=inst.engine,
                                bass_nofuse=True,
                            )
                        )
                    inst.sync_info = mybir.SyncInfo(
                        on_wait=waits[:lim],
                        on_update=list(si.on_update) if si.on_update else [],
                    )
                    changed = True
                new_list.append(inst)
            if changed:
                try:
                    bb.instructions = new_list
                except Exception:
                    insts.clear()
                    insts.extend(new_list)


def _build():
    import concourse.bass as bass
    import concourse.tile as tile
    import concourse.mybir as mybir
    from concourse.masks import make_identity

    bf16 = mybir.dt.bfloat16
    f32 = mybir.dt.float32
    EXP = mybir.ActivationFunctionType.Exp

    nc = bass.Bass("TRN2", target_bir_lowering=False, debug=False,
                   num_devices=NCORES)

    xtq_d = nc.dram_tensor("xtq", [D, S], bf16, kind="ExternalInput").ap()
    xtk_d = nc.dram_tensor("xtk", [D, S], bf16, kind="ExternalInput").ap()
    xtv_d = nc.dram_tensor("xtv", [D, S], bf16, kind="ExternalInput").ap()
    wqkv_d = nc.dram_tensor("wqkv", [D, 3 * DSL], bf16, kind="ExternalInput").ap()
    wo_d = nc.dram_tensor("wo", [DSL, D], bf16, kind="ExternalInput").ap()
    out_d = nc.dram_tensor("out", [S, D], f32, kind="ExternalOutput").ap()

    with tile.TileContext(nc) as tc:
        with (
            tc.tile_pool(name="wp", bufs=1) as wp,
            tc.tile_pool(name="xin", bufs=XIN_BUFS) as xp,
            tc.tile_pool(name="mp", bufs=1) as mp,
            tc.tile_pool(name="stexp", bufs=STEXP_BUFS) as sp,
            tc.tile_pool(name="norm", bufs=NORM_BUFS) as npl,
            tc.tile_pool(name="outst", bufs=OUTST_BUFS) as op_,
            tc.tile_pool(name="psum", bufs=1, space="PSUM") as psP,
        ):
            # ---- resident tiles + input DMA ----
            wqkv_sb = wp.tile([128, KT, 3 * DSL], bf16, tag="wqkv")
            wo_sb = wp.tile([128, 2, D], bf16, tag="wo")
            ident = wp.tile([128, 128], bf16, tag="ident")
            make_identity(nc, ident)

            xq_sb = xp.tile([128, KT, S], bf16, tag="xt")
            xk_sb = xp.tile([128, KT, S], bf16, tag="xt")
            xv_sb = xp.tile([128, KT, S], bf16, tag="xt")

            # quarter-granular loads in need order: window j of the
            # qb-major walk touches only seq-quarter j, so quarters j>0
            # stream in behind window 0's compute.
            wqkv_r = wqkv_d.rearrange("(g p) n -> p g n", p=128)
            xq_r = xtq_d.rearrange("(g p) s -> p g s", p=128)
            xk_r = xtk_d.rearrange("(g p) s -> p g s", p=128)
            xv_r = xtv_d.rearrange("(g p) s -> p g s", p=128)
            quarter = lambda t, j: t[:, :, j * 512:(j + 1) * 512]
            nc.sync.dma_start(out=wqkv_sb[:], in_=wqkv_r)
            for hh in range(2):
                nc.sync.dma_start(out=quarter(xq_sb, 0)[:, hh * 4:hh * 4 + 4, :],
                                  in_=quarter(xq_r, 0)[:, hh * 4:hh * 4 + 4, :])
            for hh in range(2):
                nc.sync.dma_start(out=quarter(xk_sb, 0)[:, hh * 4:hh * 4 + 4, :],
                                  in_=quarter(xk_r, 0)[:, hh * 4:hh * 4 + 4, :])
            nc.sync.dma_start(out=quarter(xk_sb, 1), in_=quarter(xk_r, 1))
            nc.sync.dma_start(out=quarter(xv_sb, 0), in_=quarter(xv_r, 0))
            nc.sync.dma_start(out=quarter(xk_sb, 2), in_=quarter(xk_r, 2))
            nc.sync.dma_start(out=quarter(xv_sb, 1), in_=quarter(xv_r, 1))
            nc.sync.dma_start(out=quarter(xk_sb, 3), in_=quarter(xk_r, 3))
            nc.sync.dma_start(out=quarter(xv_sb, 2), in_=quarter(xv_r, 2))
            nc.sync.dma_start(out=quarter(xv_sb, 3), in_=quarter(xv_r, 3))
            for j in range(1, 4):
                nc.sync.dma_start(out=quarter(xq_sb, j), in_=quarter(xq_r, j))
            nc.sync.dma_start(out=wo_sb[:],
                              in_=wo_d.rearrange("(g p) n -> p g n", p=128))

            # Q^T/K^T: head h on partitions (h%2)*64..+64, pair index h//2.
            # V natural [k, d] with a ones-column per head.
            QT = mp.tile([128, 2, S], bf16, tag="qt")
            KTt = mp.tile([128, 2, S], bf16, tag="kt")
            Vs_sb = mp.tile([128, SC, HPC, HD + 1], bf16, tag="vs")
            xn_sb = mp.tile([128, SC, HPC, HD], bf16, tag="xn")
            xTn_sb = mp.tile([128, 2, S], bf16, tag="xtn")
            nc.vector.memset(Vs_sb[:, :, :, HD:HD + 1], 1.0)

            def st_tile():
                t = psP.tile([128, 1024], f32, tag="ps", bufs=ST_BUFS, name="stp")
                return t

            def pp_tile(shape, dtype):
                t = psP.tile(shape, dtype, tag="pp", bufs=PP_BUFS, name="ppt")
                return t

            # ---- filler chunk builders (sub-us PE work units, "pp" psum) ----

            def qk_proj_chunks_q(t, m, quarter):
                # quarter of one (t, m) projection: 4 chunks of 2 matmuls
                xsrc, dst = (xq_sb, QT) if t == 0 else (xk_sb, KTt)
                chunks = []
                psum = {}
                for g2 in range(4):
                    def chunk(g2=g2):
                        if g2 == 0:
                            psum[0] = pp_tile([128, 512], f32)
                        pqt = psum[0]
                        for g in (2 * g2, 2 * g2 + 1):
                            nc.tensor.matmul(
                                pqt[:],
                                lhsT=wqkv_sb[:, g, t * DSL + m * 128:
                                             t * DSL + (m + 1) * 128],
                                rhs=xsrc[:, g, quarter * 512:
                                         (quarter + 1) * 512],
                                start=(g == 0), stop=(g == KT - 1),
                            )
                        if g2 == 3:
                            sl = slice(quarter * 512, (quarter + 1) * 512)
                            nc.vector.tensor_copy(dst[:, m, sl], pqt[:])
                    chunks.append(chunk)
                return chunks

            def v_proj_chunks(grp):
                # one V group as 2 gens (2 seq-chunks each) x 4 chunks
                chunks = []
                psum = {}
                for half in range(2):
                    for g2 in range(4):
                        def chunk(grp=grp, half=half, g2=g2):
                            if g2 == 0:
                                psum[0] = pp_tile([128, 512], f32)
                            psv = psum[0]
                            for g in (2 * g2, 2 * g2 + 1):
                                for j in range(2):
                                    kc = grp * 4 + half * 2 + j
                                    nc.tensor.matmul(
                                        psv[:, j * DSL:(j + 1) * DSL],
                                        lhsT=xv_sb[:, g,
                                                   kc * 128:(kc + 1) * 128],
                                        rhs=wqkv_sb[:, g, 2 * DSL:3 * DSL],
                                        start=(g == 0 and j == 0),
                                        stop=(g == KT - 1),
                                    )
                            if g2 == 3:
                                kc0 = grp * 4 + half * 2
                                nc.vector.tensor_copy(
                                    Vs_sb[:, kc0:kc0 + 2, :, 0:HD],
                                    psv[:].rearrange(
                                        "p (c h d) -> p c h d", c=2, h=HPC),
                                )
                        chunks.append(chunk)
                return chunks

            out_r = out_d.rearrange("(c p) n -> p c n", p=128)

            def wo_chunks(qg):
                # one wo_group (2 q-chunks) as 5 chunks:
                # [transposes+evict], then per (qc, n-half) matmul+evict;
                # the last chunk DMAs the assembled [128, 2, D] block out.
                state = {}

                def c_transpose():
                    xt_ps = pp_tile([128, 2, 2, 128], bf16)
                    first = True
                    for j2 in range(2):
                        qc = qg * 2 + j2
                        for g2 in range(2):
                            nc.tensor.matmul(
                                xt_ps[:, g2, j2, :],
                                lhsT=xn_sb[:, qc, 2 * g2:2 * g2 + 2, :],
                                rhs=ident[:],
                                is_transpose=True,
                                start=first, stop=True,
                            )
                            first = False
                    nc.vector.tensor_copy(
                        xTn_sb[:, :, qg * 256:(qg + 1) * 256],
                        xt_ps[:].rearrange("p a b q -> p a (b q)"),
                    )

                def c_pso(j2, n2):
                    if j2 == 0 and n2 == 0:
                        state["ost"] = op_.tile([128, 2, D], f32, tag="ost",
                                                name="ost")
                    qc = qg * 2 + j2
                    pso = pp_tile([128, 512], f32)
                    for g2 in range(2):
                        nc.tensor.matmul(
                            pso[:],
                            lhsT=xTn_sb[:, g2, qc * 128:(qc + 1) * 128],
                            rhs=wo_sb[:, g2, n2 * 512:(n2 + 1) * 512],
                            start=(g2 == 0), stop=(g2 == 1),
                        )
                    nc.vector.tensor_copy(
                        state["ost"][:, j2, n2 * 512:(n2 + 1) * 512], pso[:])
                    if j2 == 1 and n2 == 1:
                        nc.sync.dma_start(
                            out=out_r[:, qg * 2:(qg + 1) * 2, :],
                            in_=state["ost"][:])

                return [c_transpose,
                        lambda: c_pso(0, 0), lambda: c_pso(0, 1),
                        lambda: c_pso(1, 0), lambda: c_pso(1, 1)]

            def wo_tail(qg):
                # tail-only: score rotation is idle -> [128,1024] pso gens,
                # per-q-chunk transpose/evict/DMA so the chain pipelines
                for j2 in range(2):
                    qc = qg * 2 + j2
                    xt_ps = pp_tile([128, 2, 128], bf16)
                    for g2 in range(2):
                        nc.tensor.matmul(
                            xt_ps[:, g2, :],
                            lhsT=xn_sb[:, qc, 2 * g2:2 * g2 + 2, :],
                            rhs=ident[:],
                            is_transpose=True,
                            start=(g2 == 0), stop=True,
                        )
                    nc.vector.tensor_copy(
                        xTn_sb[:, :, qc * 128:(qc + 1) * 128],
                        xt_ps[:])
                    ost = op_.tile([128, 1, D], f32, tag="ost", name="ost")
                    pso = st_tile()
                    for n2 in range(D // 512):
                        for g2 in range(2):
                            nc.tensor.matmul(
                                pso[:, n2 * 512:(n2 + 1) * 512],
                                lhsT=xTn_sb[:, g2, qc * 128:(qc + 1) * 128],
                                rhs=wo_sb[:, g2, n2 * 512:(n2 + 1) * 512],
                                start=(g2 == 0), stop=(g2 == 1),
                            )
                    nc.vector.tensor_copy(ost[:, 0, :], pso[:])
                    nc.sync.dma_start(out=out_r[:, qc:qc + 1, :], in_=ost[:])

            # ---- attention pipeline ----

            def attn_head(h, qb, fillers):
                # fillers: list of per-slot lists of callables (len 8)
                hb = (h % 2) * 64
                m = h // 2
                qsl = slice(qb * QB, (qb + 1) * QB)
                x_ps = psP.tile([128, 4, HD + 1], f32, tag="px", bufs=PX_BUFS,
                                name="xps")
                pe_ts = [None] * 8

                def pv_batch(pr):
                    pe_t = pe_ts[pr]
                    for half, kc in ((0, 2 * pr), (1, 2 * pr + 1)):
                        for qs in range(4):
                            nc.tensor.matmul(
                                x_ps[:, qs, :],
                                lhsT=pe_t[:, half * 512 + qs * 128:
                                          half * 512 + (qs + 1) * 128],
                                rhs=Vs_sb[:, kc, h, :],
                                start=(pr == 0 and half == 0 and qs == 0),
                                stop=(pr == SC // 2 - 1 and half == 1),
                            )

                def st_exp(pr):
                    kc0, kc1 = 2 * pr, 2 * pr + 1
                    st_pair = st_tile()
                    nc.tensor.matmul(
                        st_pair[:, 0:512],
                        lhsT=KTt[hb:hb + 64, m, kc0 * 128:(kc0 + 1) * 128],
                        rhs=QT[hb:hb + 64, m, qsl],
                        start=True, stop=True,
                    )
                    nc.tensor.matmul(
                        st_pair[:, 512:1024],
                        lhsT=KTt[hb:hb + 64, m, kc1 * 128:(kc1 + 1) * 128],
                        rhs=QT[hb:hb + 64, m, qsl],
                        start=True, stop=True,
                    )
                    pe_t = sp.tile([128, 1024], bf16, tag="stexp", name="pet")
                    nc.scalar.activation(pe_t[:], st_pair[:], EXP, scale=0.125)
                    pe_ts[pr] = pe_t

                # scores run two slots ahead of their exp's slot so the
                # psum-WAR chain (bank freed by exp(i-3)) never paces the
                # exp stream; PV runs four slots behind, with the last four
                # batches + norm deferred into the NEXT block's fillers so
                # DMA-gated V work and the px-WAR handoff never sit in front
                # of the next block's score matmuls in PE's in-order queue.
                st_exp(0)
                st_exp(1)
                for pr in range(8):
                    if pr + 2 < 8:
                        st_exp(pr + 2)
                    if pr >= 4:
                        pv_batch(pr - 4)
                    for f in fillers[pr]:
                        f()

                def norm_fn():
                    rc = npl.tile([128, 4], f32, tag="rc", name="rc")
                    nc.vector.reciprocal(rc[:], x_ps[:, :, HD:HD + 1])
                    for qs in range(4):
                        nc.vector.tensor_scalar_mul(
                            xn_sb[:, qb * 4 + qs, h, :],
                            x_ps[:, qs, 0:HD],
                            rc[:, qs:qs + 1],
                        )

                return [lambda pr=pr: pv_batch(pr) for pr in (4, 5, 6)] + \
                    [lambda: (pv_batch(7), norm_fn())]

            def spread(chunks, nslots):
                # distribute chunks over nslots slots, front-loaded
                out = [[] for _ in range(nslots)]
                for i, c in enumerate(chunks):
                    out[i * nslots // len(chunks)].append(c)
                return out

            # ---- schedule: qb-major walk ----
            # pq[t][m][j]: 4-chunk list projecting seq-quarter j of (t, m);
            # vq[g]: 8-chunk list for V seq-group g (2 gens of 4)
            pq = [[[qk_proj_chunks_q(t, m, j) for j in range(4)]
                   for m in range(2)] for t in range(2)]
            vq = [v_proj_chunks(g) for g in range(4)]

            # upfront: quarter-0 Q/K and quarter-1 K for the m=0 pair
            # (sts are issued two slots early, so K quarters must land two
            # slots sooner than their exp's slot)
            for c in pq[0][0][0] + pq[1][0][0] + pq[1][0][1]:
                c()

            empty = [[] for _ in range(8)]
            fill = {}
            fill[(0, 0)] = [
                pq[1][0][2][0:2] + vq[0][0:4],   # Kq2 a, Vg0 gen0
                pq[1][0][2][2:4] + vq[0][4:8],
                pq[1][0][3][0:2] + vq[1][0:4],   # Kq3 a, Vg1 gen0
                pq[1][0][3][2:4] + vq[1][4:8],
                [],
                [],
                vq[2][0:4],
                vq[2][4:8],
            ]
            fill[(0, 1)] = [[], [], vq[3][0:4], vq[3][4:8]] + \
                spread(pq[1][1][0] + pq[0][1][0] + pq[1][1][1], 4)
            fill[(0, 2)] = [
                pq[1][1][2],
                pq[1][1][3][0:2], pq[1][1][3][2:4],
                pq[0][0][1][0:2], pq[0][0][1][2:4],
                [], [], [],
            ]
            fill[(0, 3)] = spread(pq[0][1][1], 8)
            # wo for q-block j needs all four heads' norms of block j:
            # schedule its groups strictly in window j+1
            # wo for q-block j reads xn of ALL its heads; norm(j,3) is a
            # deferred finisher landing in (j+1,0) slot 3, so wo chunks may
            # only occupy slots 4+ of (j+1,0) (and any slot of (j+1,1)).
            fill[(1, 0)] = [[], [], [], []] + spread(wo_chunks(0), 4)
            fill[(1, 1)] = spread(wo_chunks(1), 8)
            fill[(1, 2)] = spread(pq[0][0][2] + pq[0][1][2], 8)
            fill[(1, 3)] = []
            fill[(2, 0)] = [[], [], [], []] + spread(wo_chunks(2), 4)
            fill[(2, 1)] = spread(wo_chunks(3), 8)
            fill[(2, 2)] = spread(pq[0][0][3] + pq[0][1][3], 8)
            fill[(2, 3)] = []
            fill[(3, 0)] = [[], [], [], []] + spread(wo_chunks(4), 4)
            fill[(3, 1)] = spread(wo_chunks(5), 8)
            fill[(3, 2)] = []
            fill[(3, 3)] = []
            fins = []
            for qb in range(NQB):
                for h in range(HPC):
                    ch = fill[(qb, h)]
                    if not ch:
                        ch = [[] for _ in range(8)]
                    elif not isinstance(ch[0], list):
                        ch = spread(ch, 8)
                    ch = [list(s) for s in ch]
                    # append: same-slot fillers (e.g. V gens) must precede
                    # the finisher PV batches that read them
                    for i, f in enumerate(fins):
                        ch[i].append(f)
                    fins = attn_head(h, qb, ch)
            for f in fins:
                f()
            wo_tail(6)
            wo_tail(7)

    import concourse.mybir as mybir_mod
    _split_excess_waits(nc, mybir_mod)
    return nc


def kernel(q, k, v, mask, Wq, bq, Wk, bk, Wv, bv, Wo, bo):
    global _cached_nc, _last_result
    from concourse.bass_utils import run_bass_kernel_spmd

    if _cached_nc is None:
        _cached_nc = _build()
    nc = _cached_nc

    bf = ml_dtypes.bfloat16
    q = np.asarray(q); k = np.asarray(k); v = np.asarray(v)
    Wq = np.asarray(Wq); Wk = np.asarray(Wk); Wv = np.asarray(Wv)
    Wo = np.asarray(Wo)

    xt = {}
    for b in range(B):
        xt[("q", b)] = np.ascontiguousarray(q[b].T).astype(bf)
        xt[("k", b)] = np.ascontiguousarray(k[b].T).astype(bf)
        xt[("v", b)] = np.ascontiguousarray(v[b].T).astype(bf)

    in_maps = []
    for c in range(NCORES):
        b, hg = c // GROUPS, c % GROUPS
        sl = slice(hg * DSL, (hg + 1) * DSL)
        wqkv = np.ascontiguousarray(
            np.concatenate([Wq[:, sl], Wk[:, sl], Wv[:, sl]], axis=1)
        ).astype(bf)
        wo = np.ascontiguousarray(Wo[sl, :]).astype(bf)
        in_maps.append({
            "xtq": xt[("q", b)], "xtk": xt[("k", b)], "xtv": xt[("v", b)],
            "wqkv": wqkv, "wo": wo,
        })

    try:
        res = run_bass_kernel_spmd(nc, in_maps, list(range(NCORES)),
                                   trace=TRACE, **TRACE_KW)
    except ModuleNotFoundError:
        # no NTFF profiling hook in this axon client; run without trace
        res = run_bass_kernel_spmd(nc, in_maps, list(range(NCORES)))
    _last_result = res

    out = np.empty((B, S, D), np.float32)
    for b in range(B):
        acc = res.results[GROUPS * b]["out"].copy()
        for j in range(1, GROUPS):
            acc += res.results[GROUPS * b + j]["out"]
        out[b] = acc
    return out


# revision 3
# speedup vs baseline: 1.0088x; 1.0088x over previous
"""MultiHeadAttention (B=2, S=2048, D=1024, H=16) on 8 NeuronCores.

Sharding: data-parallel over batch (2) x tensor-parallel over heads (4 groups
of 4 heads). Core c handles batch c//4, heads (c%4)*4 .. +4. Host sums the 4
partial outputs per batch (the "all-reduce" of row-parallel Wo).

v5 = v4 + q-block-major walk + quarter-granular streaming DMA + fast tail.

Walk order is qb-major (heads inner): the first 32-slot window consumes only
seq-quarter 0 of x/K/V while quarters 1-3 stream in behind it, so the serial
~39us input DMA hides almost entirely under compute. Projections for quarter
j+1 and Wo for completed q-blocks ride as fillers inside window j. Output
DMAs issue from the otherwise-idle GpSimd queue.

Data layout (vs the duplicated-QT baseline), driven by the TimelineSim cost
model where matmul cost = out-free-size x pe_cycle and activation cost =
free-size x cycle_t:
  - P.V runs in natural layout: out[q=128, d=65] += P[k,q]^T V[k,d] -- the
    65-row output uses the full 128-partition q dim, halving PV tensor time
    relative to the baseline's [65, 512] orientation,
  - softmax denominators ride along as V's ones-column (column 64 of the PV
    accumulator); normalization is a per-partition reciprocal + tensor_scalar
    multiply (no PE broadcast matmul, no cross-partition work),
  - Wo consumes x^T[d, q], produced by cheap PE transpose matmuls of the
    normalized [q, d] tiles,
  - Q^T/K^T are NOT duplicated across row-group halves: head h lives on
    partitions (h%2)*64..+64 straight out of its projection eviction.

Issue order: the exp stream on ScalarE is the pacer (~1.04us per 128x1024
tile, 128 tiles). Per pr-slot the PE stream is [st_pair(pr); PV(pr-1);
filler], so the score matmuls for the NEXT exp are always ahead of the PV
batch that blocks on the CURRENT exp, and projection/Wo matmuls are injected
in sub-us chunks to fill PE's per-slot slack without starving ScalarE.

PSUM (8 banks): "ps" 3x[128,1024]f32 (6 banks) for the score tiles and the
pre-attention projections; "pp" 1x 1 bank for interleaved filler psums
([128,512]f32 projections / wo, [128,2,2,128]bf16 transposes) -- fillers hold
their psum across slots, so they must NOT share the st rotation; "px" 1 bank
for the PV accumulator.
"""

import numpy as np
import ml_dtypes

B, S, D, H = 2, 2048, 1024, 16
HD = 64
NCORES = 8
GROUPS = 4            # head groups (tensor-parallel degree per batch)
HPC = H // GROUPS     # 4 heads per core
DSL = HPC * HD        # 256: per-core slice of D
KT = D // 128         # 8 contraction tiles for projections
SC = S // 128         # 16 sequence chunks
QB = 512              # q-block for attention phase
NQB = S // QB         # 4

_cached_nc = None
TRACE = False
TRACE_KW = {}
_last_result = None

# scheduling tunables
STEXP_BUFS = 14
NORM_BUFS = 2
OUTST_BUFS = 2
XIN_BUFS = 3
ST_BUFS = 3           # [128,1024] f32 score tiles (2 banks each)
PP_BUFS = 1           # 1-bank filler psums
PX_BUFS = 1           # [128,4,65] f32 PV accumulators (1 bank)


def _split_excess_waits(nc, mybir, max_waits=1):
    # walrus (core_v3) rejects instructions carrying more sync waits than the
    # ISA struct holds; hoist extras onto preceding same-engine NoOps.
    for fn in nc.m.functions:
        for bb in fn.blocks:
            insts = bb.instructions
            new_list = []
            changed = False
            for inst in insts:
                si = inst.sync_info
                waits = list(si.on_wait) if si and si.on_wait else []
                lim = 2 if isinstance(inst, mybir.InstEventSemaphore) else max_waits
                if len(waits) > lim:
                    for j, w in enumerate(waits[lim:]):
                        new_list.append(
                            mybir.InstNoOp(
                                name=f"{inst.name}-wsplit{j}",
                                sync_info=mybir.SyncInfo(on_wait=[w], on_update=[]),
                                engine=inst.engine,
                                bass_nofuse=True,
                            )
                        )
                    inst.sync_info = mybir.SyncInfo(
                        on_wait=waits[:lim],
                        on_update=list(si.on_update) if si.on_update else [],
                    )
                    changed = True
                new_list.append(inst)
            if changed:
                try:
                    bb.instructions = new_list
                except Exception:
                    insts.clear()
                    insts.extend(new_list)


def _build():
    import concourse.bass as bass
    import concourse.tile as tile
    import concourse.mybir as mybir
    from concourse.masks import make_identity

    bf16 = mybir.dt.bfloat16
    f32 = mybir.dt.float32
    EXP = mybir.ActivationFunctionType.Exp

    nc = bass.Bass("TRN2", target_bir_lowering=False, debug=False,
                   num_devices=NCORES)

    xtq_d = nc.dram_tensor("xtq", [D, S], bf16, kind="ExternalInput").ap()
    xtk_d = nc.dram_tensor("xtk", [D, S], bf16, kind="ExternalInput").ap()
    xtv_d = nc.dram_tensor("xtv", [D, S], bf16, kind="ExternalInput").ap()
    wqkv_d = nc.dram_tensor("wqkv", [D, 3 * DSL], bf16, kind="ExternalInput").ap()
    wo_d = nc.dram_tensor("wo", [DSL, D], bf16, kind="ExternalInput").ap()
    out_d = nc.dram_tensor("out", [S, D], f32, kind="ExternalOutput").ap()

    with tile.TileContext(nc) as tc:
        with (
            tc.tile_pool(name="wp", bufs=1) as wp,
            tc.tile_pool(name="xin", bufs=XIN_BUFS) as xp,
            tc.tile_pool(name="mp", bufs=1) as mp,
            tc.tile_pool(name="stexp", bufs=STEXP_BUFS) as sp,
            tc.tile_pool(name="norm", bufs=NORM_BUFS) as npl,
            tc.tile_pool(name="outst", bufs=OUTST_BUFS) as op_,
            tc.tile_pool(name="psum", bufs=1, space="PSUM") as psP,
        ):
            # ---- resident tiles + input DMA ----
            wqkv_sb = wp.tile([128, KT, 3 * DSL], bf16, tag="wqkv")
            wo_sb = wp.tile([128, 2, D], bf16, tag="wo")
            ident = wp.tile([128, 128], bf16, tag="ident")
            make_identity(nc, ident)

            xq_sb = xp.tile([128, KT, S], bf16, tag="xt")
            xk_sb = xp.tile([128, KT, S], bf16, tag="xt")
            xv_sb = xp.tile([128, KT, S], bf16, tag="xt")

            # quarter-granular loads in need order: window j of the
            # qb-major walk touches only seq-quarter j, so quarters j>0
            # stream in behind window 0's compute.
            wqkv_r = wqkv_d.rearrange("(g p) n -> p g n", p=128)
            xq_r = xtq_d.rearrange("(g p) s -> p g s", p=128)
            xk_r = xtk_d.rearrange("(g p) s -> p g s", p=128)
            xv_r = xtv_d.rearrange("(g p) s -> p g s", p=128)
            quarter = lambda t, j: t[:, :, j * 512:(j + 1) * 512]
            nc.sync.dma_start(out=wqkv_sb[:], in_=wqkv_r)
            for hh in range(2):
                nc.sync.dma_start(out=quarter(xq_sb, 0)[:, hh * 4:hh * 4 + 4, :],
                                  in_=quarter(xq_r, 0)[:, hh * 4:hh * 4 + 4, :])
            for hh in range(2):
                nc.sync.dma_start(out=quarter(xk_sb, 0)[:, hh * 4:hh * 4 + 4, :],
                                  in_=quarter(xk_r, 0)[:, hh * 4:hh * 4 + 4, :])
            nc.sync.dma_start(out=quarter(xk_sb, 1), in_=quarter(xk_r, 1))
            nc.sync.dma_start(out=quarter(xv_sb, 0), in_=quarter(xv_r, 0))
            nc.sync.dma_start(out=quarter(xk_sb, 2), in_=quarter(xk_r, 2))
            nc.sync.dma_start(out=quarter(xv_sb, 1), in_=quarter(xv_r, 1))
            nc.sync.dma_start(out=quarter(xk_sb, 3), in_=quarter(xk_r, 3))
            nc.sync.dma_start(out=quarter(xv_sb, 2), in_=quarter(xv_r, 2))
            nc.sync.dma_start(out=quarter(xv_sb, 3), in_=quarter(xv_r, 3))
            for j in range(1, 4):
                nc.sync.dma_start(out=quarter(xq_sb, j), in_=quarter(xq_r, j))
            nc.sync.dma_start(out=wo_sb[:],
                              in_=wo_d.rearrange("(g p) n -> p g n", p=128))

            # Q^T/K^T: head h on partitions (h%2)*64..+64, pair index h//2.
            # V natural [k, d] with a ones-column per head.
            QT = mp.tile([128, 2, S], bf16, tag="qt")
            KTt = mp.tile([128, 2, S], bf16, tag="kt")
            Vs_sb = mp.tile([128, SC, HPC, HD + 1], bf16, tag="vs")
            xn_sb = mp.tile([128, SC, HPC, HD], bf16, tag="xn")
            xTn_sb = mp.tile([128, 2, S], bf16, tag="xtn")
            nc.vector.memset(Vs_sb[:, :, :, HD:HD + 1], 1.0)

            def st_tile():
                t = psP.tile([128, 1024], f32, tag="ps", bufs=ST_BUFS, name="stp")
                return t

            def pp_tile(shape, dtype):
                t = psP.tile(shape, dtype, tag="pp", bufs=PP_BUFS, name="ppt")
                return t

            # ---- filler chunk builders (sub-us PE work units, "pp" psum) ----

            def qk_proj_chunks_q(t, m, quarter):
                # quarter of one (t, m) projection: 4 chunks of 2 matmuls
                xsrc, dst = (xq_sb, QT) if t == 0 else (xk_sb, KTt)
                chunks = []
                psum = {}
                for g2 in range(4):
                    def chunk(g2=g2):
                        if g2 == 0:
                            psum[0] = pp_tile([128, 512], f32)
                        pqt = psum[0]
                        for g in (2 * g2, 2 * g2 + 1):
                            nc.tensor.matmul(
                                pqt[:],
                                lhsT=wqkv_sb[:, g, t * DSL + m * 128:
                                             t * DSL + (m + 1) * 128],
                                rhs=xsrc[:, g, quarter * 512:
                                         (quarter + 1) * 512],
                                start=(g == 0), stop=(g == KT - 1),
                            )
                        if g2 == 3:
                            sl = slice(quarter * 512, (quarter + 1) * 512)
                            nc.vector.tensor_copy(dst[:, m, sl], pqt[:])
                    chunks.append(chunk)
                return chunks

            def v_proj_chunks(grp):
                # one V group as 2 gens (2 seq-chunks each) x 4 chunks
                chunks = []
                psum = {}
                for half in range(2):
                    for g2 in range(4):
                        def chunk(grp=grp, half=half, g2=g2):
                            if g2 == 0:
                                psum[0] = pp_tile([128, 512], f32)
                            psv = psum[0]
                            for g in (2 * g2, 2 * g2 + 1):
                                for j in range(2):
                                    kc = grp * 4 + half * 2 + j
                                    nc.tensor.matmul(
                                        psv[:, j * DSL:(j + 1) * DSL],
                                        lhsT=xv_sb[:, g,
                                                   kc * 128:(kc + 1) * 128],
                                        rhs=wqkv_sb[:, g, 2 * DSL:3 * DSL],
                                        start=(g == 0 and j == 0),
                                        stop=(g == KT - 1),
                                    )
                            if g2 == 3:
                                kc0 = grp * 4 + half * 2
                                nc.vector.tensor_copy(
                                    Vs_sb[:, kc0:kc0 + 2, :, 0:HD],
                                    psv[:].rearrange(
                                        "p (c h d) -> p c h d", c=2, h=HPC),
                                )
                        chunks.append(chunk)
                return chunks

            out_r = out_d.rearrange("(c p) n -> p c n", p=128)

            def wo_chunks(qg):
                # one wo_group (2 q-chunks) as 5 chunks:
                # [transposes+evict], then per (qc, n-half) matmul+evict;
                # the last chunk DMAs the assembled [128, 2, D] block out.
                state = {}

                def c_transpose():
                    xt_ps = pp_tile([128, 2, 2, 128], bf16)
                    first = True
                    for j2 in range(2):
                        qc = qg * 2 + j2
                        for g2 in range(2):
                            nc.tensor.matmul(
                                xt_ps[:, g2, j2, :],
                                lhsT=xn_sb[:, qc, 2 * g2:2 * g2 + 2, :],
                                rhs=ident[:],
                                is_transpose=True,
                                start=first, stop=True,
                            )
                            first = False
                    nc.vector.tensor_copy(
                        xTn_sb[:, :, qg * 256:(qg + 1) * 256],
                        xt_ps[:].rearrange("p a b q -> p a (b q)"),
                    )

                def c_pso(j2, n2):
                    if j2 == 0 and n2 == 0:
                        state["ost"] = op_.tile([128, 2, D], f32, tag="ost",
                                                name="ost")
                    qc = qg * 2 + j2
                    pso = pp_tile([128, 512], f32)
                    for g2 in range(2):
                        nc.tensor.matmul(
                            pso[:],
                            lhsT=xTn_sb[:, g2, qc * 128:(qc + 1) * 128],
                            rhs=wo_sb[:, g2, n2 * 512:(n2 + 1) * 512],
                            start=(g2 == 0), stop=(g2 == 1),
                        )
                    nc.vector.tensor_copy(
                        state["ost"][:, j2, n2 * 512:(n2 + 1) * 512], pso[:])
                    if j2 == 1 and n2 == 1:
                        nc.sync.dma_start(
                            out=out_r[:, qg * 2:(qg + 1) * 2, :],
                            in_=state["ost"][:])

                return [c_transpose,
                        lambda: c_pso(0, 0), lambda: c_pso(0, 1),
                        lambda: c_pso(1, 0), lambda: c_pso(1, 1)]

            def wo_tail(qg):
                # tail-only: score rotation is idle -> [128,1024] pso gens,
                # per-q-chunk transpose/evict/DMA so the chain pipelines
                for j2 in range(2):
                    qc = qg * 2 + j2
                    xt_ps = pp_tile([128, 2, 128], bf16)
                    for g2 in range(2):
                        nc.tensor.matmul(
                            xt_ps[:, g2, :],
                            lhsT=xn_sb[:, qc, 2 * g2:2 * g2 + 2, :],
                            rhs=ident[:],
                            is_transpose=True,
                            start=(g2 == 0), stop=True,
                        )
                    nc.vector.tensor_copy(
                        xTn_sb[:, :, qc * 128:(qc + 1) * 128],
                        xt_ps[:])
                    ost = op_.tile([128, 1, D], f32, tag="ost", name="ost")
                    pso = st_tile()
                    for n2 in range(D // 512):
                        for g2 in range(2):
                            nc.tensor.matmul(
                                pso[:, n2 * 512:(n2 + 1) * 512],
                                lhsT=xTn_sb[:, g2, qc * 128:(qc + 1) * 128],
                                rhs=wo_sb[:, g2, n2 * 512:(n2 + 1) * 512],
                                start=(g2 == 0), stop=(g2 == 1),
                            )
                    # alternate evict engine: ScalarE is idle after the last
                    # exp, halving the DVE-gated tail cadence
                    if j2 == 0:
                        nc.scalar.copy(ost[:, 0, :], pso[:])
                    else:
                        nc.vector.tensor_copy(ost[:, 0, :], pso[:])
                    nc.sync.dma_start(out=out_r[:, qc:qc + 1, :], in_=ost[:])

            # ---- attention pipeline ----

            def attn_head(h, qb, fillers):
                # fillers: list of per-slot lists of callables (len 8)
                hb = (h % 2) * 64
                m = h // 2
                qsl = slice(qb * QB, (qb + 1) * QB)
                x_ps = psP.tile([128, 4, HD + 1], f32, tag="px", bufs=PX_BUFS,
                                name="xps")
                pe_ts = [None] * 8

                def pv_batch(pr):
                    pe_t = pe_ts[pr]
                    for half, kc in ((0, 2 * pr), (1, 2 * pr + 1)):
                        for qs in range(4):
                            nc.tensor.matmul(
                                x_ps[:, qs, :],
                                lhsT=pe_t[:, half * 512 + qs * 128:
                                          half * 512 + (qs + 1) * 128],
                                rhs=Vs_sb[:, kc, h, :],
                                start=(pr == 0 and half == 0 and qs == 0),
                                stop=(pr == SC // 2 - 1 and half == 1),
                            )

                def st_exp(pr):
                    kc0, kc1 = 2 * pr, 2 * pr + 1
                    st_pair = st_tile()
                    nc.tensor.matmul(
                        st_pair[:, 0:512],
                        lhsT=KTt[hb:hb + 64, m, kc0 * 128:(kc0 + 1) * 128],
                        rhs=QT[hb:hb + 64, m, qsl],
                        start=True, stop=True,
                    )
                    nc.tensor.matmul(
                        st_pair[:, 512:1024],
                        lhsT=KTt[hb:hb + 64, m, kc1 * 128:(kc1 + 1) * 128],
                        rhs=QT[hb:hb + 64, m, qsl],
                        start=True, stop=True,
                    )
                    pe_t = sp.tile([128, 1024], bf16, tag="stexp", name="pet")
                    nc.scalar.activation(pe_t[:], st_pair[:], EXP, scale=0.125)
                    pe_ts[pr] = pe_t

                # scores run two slots ahead of their exp's slot so the
                # psum-WAR chain (bank freed by exp(i-3)) never paces the
                # exp stream; PV runs four slots behind, with the last four
                # batches + norm deferred into the NEXT block's fillers so
                # DMA-gated V work and the px-WAR handoff never sit in front
                # of the next block's score matmuls in PE's in-order queue.
                st_exp(0)
                st_exp(1)
                for pr in range(8):
                    if pr + 2 < 8:
                        st_exp(pr + 2)
                    if pr >= 4:
                        pv_batch(pr - 4)
                    for f in fillers[pr]:
                        f()

                def norm_fn():
                    rc = npl.tile([128, 4], f32, tag="rc", name="rc")
                    nc.vector.reciprocal(rc[:], x_ps[:, :, HD:HD + 1])
                    for qs in range(4):
                        nc.vector.tensor_scalar_mul(
                            xn_sb[:, qb * 4 + qs, h, :],
                            x_ps[:, qs, 0:HD],
                            rc[:, qs:qs + 1],
                        )

                return [lambda pr=pr: pv_batch(pr) for pr in (4, 5, 6)] + \
                    [lambda: (pv_batch(7), norm_fn())]

            def spread(chunks, nslots):
                # distribute chunks over nslots slots, front-loaded
                out = [[] for _ in range(nslots)]
                for i, c in enumerate(chunks):
                    out[i * nslots // len(chunks)].append(c)
                return out

            # ---- schedule: qb-major walk ----
            # pq[t][m][j]: 4-chunk list projecting seq-quarter j of (t, m);
            # vq[g]: 8-chunk list for V seq-group g (2 gens of 4)
            pq = [[[qk_proj_chunks_q(t, m, j) for j in range(4)]
                   for m in range(2)] for t in range(2)]
            vq = [v_proj_chunks(g) for g in range(4)]

            # upfront: quarter-0 Q/K and quarter-1 K for the m=0 pair
            # (sts are issued two slots early, so K quarters must land two
            # slots sooner than their exp's slot)
            for c in pq[0][0][0] + pq[1][0][0] + pq[1][0][1]:
                c()

            empty = [[] for _ in range(8)]
            fill = {}
            # walk: (qb0,qb1) x m0 heads, then their m1 heads, then
            # (qb2,qb3) likewise. m=1 projections leave window 0 entirely;
            # each qb finishes 2+ blocks before its wo chunks are scheduled.
            walk = [(0, 0), (0, 1), (1, 0), (1, 1),
                    (0, 2), (0, 3), (1, 2), (1, 3),
                    (2, 0), (2, 1), (3, 0), (3, 1),
                    (2, 2), (2, 3), (3, 2), (3, 3)]
            fill[(0, 0)] = [
                vq[0][0:4] + pq[1][0][2][0:2],   # Vg0 (xv0@16), Kq2 (18.9)
                vq[0][4:8] + pq[1][0][2][2:4],
                vq[1][0:4],                      # Vg1 gen0 (xv1@21.8)
                pq[1][0][3],                     # Kq3 (24.7, hard st(6) dl)
                vq[1][4:8],
                [],
                [],
                vq[2][0:4],                      # Vg2 gen0 (27.6)
            ]
            fill[(0, 1)] = [
                vq[2][4:8],
                vq[3][0:4],                      # Vg3 (xv3@30.5)
                vq[3][4:8],
                [],
                pq[0][0][1][0:2],                # Qm0 q1 (xq1@33.4, (1,0) dl)
                pq[0][0][1][2:4],
                pq[1][1][0][0:2],                # Km1 q0
                pq[1][1][0][2:4],
            ]
            fill[(1, 0)] = spread(pq[1][1][1] + pq[1][1][2], 8)
            fill[(1, 1)] = spread(pq[1][1][3] + pq[0][1][0] + pq[0][1][1], 8)
            fill[(0, 2)] = []
            fill[(0, 3)] = []
            fill[(1, 2)] = [pq[0][0][2][0:2], pq[0][0][2][2:4], [], []] + \
                spread(wo_chunks(0), 4)
            fill[(1, 3)] = spread(wo_chunks(1), 8)
            fill[(2, 0)] = [pq[0][0][3][0:2], pq[0][0][3][2:4], [], []] + \
                spread(wo_chunks(2), 4)
            fill[(2, 1)] = spread(wo_chunks(3), 8)
            fill[(3, 0)] = [pq[0][1][2][0:2], pq[0][1][2][2:4], [], []] + [
                [], [], [], []]
            fill[(3, 1)] = [pq[0][1][3][0:2], pq[0][1][3][2:4], [], []] + [
                [], [], [], []]
            fill[(2, 2)] = []
            fill[(2, 3)] = []
            fill[(3, 2)] = [[], [], [], []] + spread(wo_chunks(4), 4)
            fill[(3, 3)] = spread(wo_chunks(5), 5) + [[], [], []]
            fins = []
            for qb, h in walk:
                ch = fill[(qb, h)]
                if not ch:
                    ch = [[] for _ in range(8)]
                elif not isinstance(ch[0], list):
                    ch = spread(ch, 8)
                ch = [list(s) for s in ch]
                # append: same-slot fillers (e.g. V gens) must precede
                # the finisher PV batches that read them
                for i, f in enumerate(fins):
                    ch[i].append(f)
                fins = attn_head(h, qb, ch)
            for f in fins:
                f()
            wo_tail(6)
            wo_tail(7)

    import concourse.mybir as mybir_mod
    _split_excess_waits(nc, mybir_mod)
    return nc


def kernel(q, k, v, mask, Wq, bq, Wk, bk, Wv, bv, Wo, bo):
    global _cached_nc, _last_result
    from concourse.bass_utils import run_bass_kernel_spmd

    if _cached_nc is None:
        _cached_nc = _build()
    nc = _cached_nc

    bf = ml_dtypes.bfloat16
    q = np.asarray(q); k = np.asarray(k); v = np.asarray(v)
    Wq = np.asarray(Wq); Wk = np.asarray(Wk); Wv = np.asarray(Wv)
    Wo = np.asarray(Wo)

    xt = {}
    for b in range(B):
        xt[("q", b)] = np.ascontiguousarray(q[b].T).astype(bf)
        xt[("k", b)] = np.ascontiguousarray(k[b].T).astype(bf)
        xt[("v", b)] = np.ascontiguousarray(v[b].T).astype(bf)

    in_maps = []
    for c in range(NCORES):
        b, hg = c // GROUPS, c % GROUPS
        sl = slice(hg * DSL, (hg + 1) * DSL)
        wqkv = np.ascontiguousarray(
            np.concatenate([Wq[:, sl], Wk[:, sl], Wv[:, sl]], axis=1)
        ).astype(bf)
        wo = np.ascontiguousarray(Wo[sl, :]).astype(bf)
        in_maps.append({
            "xtq": xt[("q", b)], "xtk": xt[("k", b)], "xtv": xt[("v", b)],
            "wqkv": wqkv, "wo": wo,
        })

    try:
        res = run_bass_kernel_spmd(nc, in_maps, list(range(NCORES)),
                                   trace=TRACE, **TRACE_KW)
    except ModuleNotFoundError:
        # no NTFF profiling hook in this axon client; run without trace
        res = run_bass_kernel_spmd(nc, in_maps, list(range(NCORES)))
    _last_result = res

    out = np.empty((B, S, D), np.float32)
    for b in range(B):
        acc = res.results[GROUPS * b]["out"].copy()
        for j in range(1, GROUPS):
            acc += res.results[GROUPS * b + j]["out"]
        out[b] = acc
    return out


# revision 5
# speedup vs baseline: 1.0132x; 1.0044x over previous
"""MultiHeadAttention (B=2, S=2048, D=1024, H=16) on 8 NeuronCores.

Sharding: data-parallel over batch (2) x tensor-parallel over heads (4 groups
of 4 heads). Core c handles batch c//4, heads (c%4)*4 .. +4. Host sums the 4
partial outputs per batch (the "all-reduce" of row-parallel Wo).

v5 = v4 + q-block-major walk + quarter-granular streaming DMA + fast tail.

Walk order is qb-major (heads inner): the first 32-slot window consumes only
seq-quarter 0 of x/K/V while quarters 1-3 stream in behind it, so the serial
~39us input DMA hides almost entirely under compute. Projections for quarter
j+1 and Wo for completed q-blocks ride as fillers inside window j. Output
DMAs issue from the otherwise-idle GpSimd queue.

Data layout (vs the duplicated-QT baseline), driven by the TimelineSim cost
model where matmul cost = out-free-size x pe_cycle and activation cost =
free-size x cycle_t:
  - P.V runs in natural layout: out[q=128, d=65] += P[k,q]^T V[k,d] -- the
    65-row output uses the full 128-partition q dim, halving PV tensor time
    relative to the baseline's [65, 512] orientation,
  - softmax denominators ride along as V's ones-column (column 64 of the PV
    accumulator); normalization is a per-partition reciprocal + tensor_scalar
    multiply (no PE broadcast matmul, no cross-partition work),
  - Wo consumes x^T[d, q], produced by cheap PE transpose matmuls of the
    normalized [q, d] tiles,
  - Q^T/K^T are NOT duplicated across row-group halves: head h lives on
    partitions (h%2)*64..+64 straight out of its projection eviction.

Issue order: the exp stream on ScalarE is the pacer (~1.04us per 128x1024
tile, 128 tiles). Per pr-slot the PE stream is [st_pair(pr); PV(pr-1);
filler], so the score matmuls for the NEXT exp are always ahead of the PV
batch that blocks on the CURRENT exp, and projection/Wo matmuls are injected
in sub-us chunks to fill PE's per-slot slack without starving ScalarE.

PSUM (8 banks): "ps" 3x[128,1024]f32 (6 banks) for the score tiles and the
pre-attention projections; "pp" 1x 1 bank for interleaved filler psums
([128,512]f32 projections / wo, [128,2,2,128]bf16 transposes) -- fillers hold
their psum across slots, so they must NOT share the st rotation; "px" 1 bank
for the PV accumulator.
"""

import numpy as np
import ml_dtypes

B, S, D, H = 2, 2048, 1024, 16
HD = 64
NCORES = 8
GROUPS = 4            # head groups (tensor-parallel degree per batch)
HPC = H // GROUPS     # 4 heads per core
DSL = HPC * HD        # 256: per-core slice of D
KT = D // 128         # 8 contraction tiles for projections
SC = S // 128         # 16 sequence chunks
QB = 512              # q-block for attention phase
NQB = S // QB         # 4

_cached_nc = None
TRACE = False
TRACE_KW = {}
_last_result = None

# scheduling tunables
STEXP_BUFS = 13
NORM_BUFS = 2
OUTST_BUFS = 3
XIN_BUFS = 3
ST_BUFS = 3           # [128,1024] f32 score tiles (2 banks each)
PP_BUFS = 1           # 1-bank filler psums
PX_BUFS = 1           # [128,4,65] f32 PV accumulators (1 bank)


def _split_excess_waits(nc, mybir, max_waits=1):
    # walrus (core_v3) rejects instructions carrying more sync waits than the
    # ISA struct holds; hoist extras onto preceding same-engine NoOps.
    #
    # Wait ordering matters for the pipeline: a wait kept ON the instruction
    # is evaluated in the engine's wait queue (pre-staged, overlaps the
    # running instruction), while a wait on a NoOp blocks the SEQ. Keep the
    # real cross-engine gates on the instruction and hoist same-engine
    # counter waits -- those are satisfied by in-order execution long before
    # the instruction reaches the engine, so their NoOps never block.
    sem_engines = {}
    for fn in nc.m.functions:
        for bb in fn.blocks:
            for inst in bb.instructions:
                si = inst.sync_info
                if si and si.on_update:
                    for u in si.on_update:
                        sem_engines.setdefault(u.id, set()).add(inst.engine)
    for fn in nc.m.functions:
        for bb in fn.blocks:
            insts = bb.instructions
            new_list = []
            changed = False
            for inst in insts:
                si = inst.sync_info
                waits = list(si.on_wait) if si and si.on_wait else []
                lim = 2 if isinstance(inst, mybir.InstEventSemaphore) else max_waits
                if len(waits) > lim:
                    own = [w for w in waits
                           if sem_engines.get(w.id) == {inst.engine}]
                    cross = [w for w in waits
                             if sem_engines.get(w.id) != {inst.engine}]
                    waits = cross + own
                    for j, w in enumerate(waits[lim:]):
                        new_list.append(
                            mybir.InstNoOp(
                                name=f"{inst.name}-wsplit{j}",
                                sync_info=mybir.SyncInfo(on_wait=[w], on_update=[]),
                                engine=inst.engine,
                                bass_nofuse=True,
                            )
                        )
                    inst.sync_info = mybir.SyncInfo(
                        on_wait=waits[:lim],
                        on_update=list(si.on_update) if si.on_update else [],
                    )
                    changed = True
                new_list.append(inst)
            if changed:
                try:
                    bb.instructions = new_list
                except Exception:
                    insts.clear()
                    insts.extend(new_list)


def _build():
    import concourse.bass as bass
    import concourse.tile as tile
    import concourse.mybir as mybir
    from concourse.masks import make_identity

    bf16 = mybir.dt.bfloat16
    f32 = mybir.dt.float32
    EXP = mybir.ActivationFunctionType.Exp

    nc = bass.Bass("TRN2", target_bir_lowering=False, debug=False,
                   num_devices=NCORES)

    xtq_d = nc.dram_tensor("xtq", [D, S], bf16, kind="ExternalInput").ap()
    xtk_d = nc.dram_tensor("xtk", [D, S], bf16, kind="ExternalInput").ap()
    xtv_d = nc.dram_tensor("xtv", [D, S], bf16, kind="ExternalInput").ap()
    wqkv_d = nc.dram_tensor("wqkv", [D, 3 * DSL], bf16, kind="ExternalInput").ap()
    wo_d = nc.dram_tensor("wo", [DSL, D], bf16, kind="ExternalInput").ap()
    out_d = nc.dram_tensor("out", [S, D], f32, kind="ExternalOutput").ap()

    with tile.TileContext(nc) as tc:
        with (
            tc.tile_pool(name="wp", bufs=1) as wp,
            tc.tile_pool(name="xin", bufs=XIN_BUFS) as xp,
            tc.tile_pool(name="mp", bufs=1) as mp,
            tc.tile_pool(name="stexp", bufs=STEXP_BUFS) as sp,
            tc.tile_pool(name="norm", bufs=NORM_BUFS) as npl,
            tc.tile_pool(name="outst", bufs=OUTST_BUFS) as op_,
            tc.tile_pool(name="psum", bufs=1, space="PSUM") as psP,
        ):
            # ---- resident tiles + input DMA ----
            wqkv_sb = wp.tile([128, KT, 3 * DSL], bf16, tag="wqkv")
            wo_sb = wp.tile([128, 2, D], bf16, tag="wo")
            ident = wp.tile([128, 128], bf16, tag="ident")
            make_identity(nc, ident)

            xq_sb = xp.tile([128, KT, S], bf16, tag="xt")
            xk_sb = xp.tile([128, KT, S], bf16, tag="xt")
            xv_sb = xp.tile([128, KT, S], bf16, tag="xt")

            # quarter-granular loads in need order: window j of the
            # qb-major walk touches only seq-quarter j, so quarters j>0
            # stream in behind window 0's compute.
            wqkv_r = wqkv_d.rearrange("(g p) n -> p g n", p=128)
            xq_r = xtq_d.rearrange("(g p) s -> p g s", p=128)
            xk_r = xtk_d.rearrange("(g p) s -> p g s", p=128)
            xv_r = xtv_d.rearrange("(g p) s -> p g s", p=128)
            quarter = lambda t, j: t[:, :, j * 512:(j + 1) * 512]
            nc.sync.dma_start(out=wqkv_sb[:], in_=wqkv_r)
            for hh in range(2):
                nc.sync.dma_start(out=quarter(xq_sb, 0)[:, hh * 4:hh * 4 + 4, :],
                                  in_=quarter(xq_r, 0)[:, hh * 4:hh * 4 + 4, :])
            for hh in range(2):
                nc.sync.dma_start(out=quarter(xk_sb, 0)[:, hh * 4:hh * 4 + 4, :],
                                  in_=quarter(xk_r, 0)[:, hh * 4:hh * 4 + 4, :])
            nc.sync.dma_start(out=quarter(xk_sb, 1), in_=quarter(xk_r, 1))
            nc.sync.dma_start(out=quarter(xv_sb, 0), in_=quarter(xv_r, 0))
            nc.sync.dma_start(out=quarter(xk_sb, 2), in_=quarter(xk_r, 2))
            nc.sync.dma_start(out=quarter(xv_sb, 1), in_=quarter(xv_r, 1))
            nc.sync.dma_start(out=quarter(xk_sb, 3), in_=quarter(xk_r, 3))
            nc.sync.dma_start(out=quarter(xv_sb, 2), in_=quarter(xv_r, 2))
            nc.sync.dma_start(out=quarter(xv_sb, 3), in_=quarter(xv_r, 3))
            for j in range(1, 4):
                nc.sync.dma_start(out=quarter(xq_sb, j), in_=quarter(xq_r, j))
            nc.sync.dma_start(out=wo_sb[:],
                              in_=wo_d.rearrange("(g p) n -> p g n", p=128))

            # Q^T/K^T: head h on partitions (h%2)*64..+64, pair index h//2.
            # V natural [k, d] with a ones-column per head.
            QT = mp.tile([128, 2, S], bf16, tag="qt")
            KTt = mp.tile([128, 2, S], bf16, tag="kt")
            Vs_sb = mp.tile([128, SC, HPC, HD + 1], bf16, tag="vs")
            xn_sb = mp.tile([128, SC, HPC, HD], bf16, tag="xn")
            xTn_sb = mp.tile([128, 2, S], bf16, tag="xtn")
            nc.vector.memset(Vs_sb[:, :, :, HD:HD + 1], 1.0)

            def st_tile():
                t = psP.tile([128, 1024], f32, tag="ps", bufs=ST_BUFS, name="stp")
                return t

            def pp_tile(shape, dtype):
                t = psP.tile(shape, dtype, tag="pp", bufs=PP_BUFS, name="ppt")
                return t

            # ---- filler chunk builders (sub-us PE work units, "pp" psum) ----

            def qk_proj_chunks_q(t, m, quarter):
                # quarter of one (t, m) projection: 4 chunks of 2 matmuls
                xsrc, dst = (xq_sb, QT) if t == 0 else (xk_sb, KTt)
                chunks = []
                psum = {}
                for g2 in range(4):
                    def chunk(g2=g2):
                        if g2 == 0:
                            psum[0] = pp_tile([128, 512], f32)
                        pqt = psum[0]
                        for g in (2 * g2, 2 * g2 + 1):
                            nc.tensor.matmul(
                                pqt[:],
                                lhsT=wqkv_sb[:, g, t * DSL + m * 128:
                                             t * DSL + (m + 1) * 128],
                                rhs=xsrc[:, g, quarter * 512:
                                         (quarter + 1) * 512],
                                start=(g == 0), stop=(g == KT - 1),
                            )
                        if g2 == 3:
                            sl = slice(quarter * 512, (quarter + 1) * 512)
                            nc.vector.tensor_copy(dst[:, m, sl], pqt[:])
                    chunks.append(chunk)
                return chunks

            def v_proj_chunks(grp):
                # one V group as 2 gens (2 seq-chunks each) x 4 chunks
                chunks = []
                psum = {}
                for half in range(2):
                    for g2 in range(4):
                        def chunk(grp=grp, half=half, g2=g2):
                            if g2 == 0:
                                psum[0] = pp_tile([128, 512], f32)
                            psv = psum[0]
                            for g in (2 * g2, 2 * g2 + 1):
                                for j in range(2):
                                    kc = grp * 4 + half * 2 + j
                                    nc.tensor.matmul(
                                        psv[:, j * DSL:(j + 1) * DSL],
                                        lhsT=xv_sb[:, g,
                                                   kc * 128:(kc + 1) * 128],
                                        rhs=wqkv_sb[:, g, 2 * DSL:3 * DSL],
                                        start=(g == 0 and j == 0),
                                        stop=(g == KT - 1),
                                    )
                            if g2 == 3:
                                kc0 = grp * 4 + half * 2
                                nc.vector.tensor_copy(
                                    Vs_sb[:, kc0:kc0 + 2, :, 0:HD],
                                    psv[:].rearrange(
                                        "p (c h d) -> p c h d", c=2, h=HPC),
                                )
                        chunks.append(chunk)
                return chunks

            out_r = out_d.rearrange("(c p) n -> p c n", p=128)

            def wo_chunks(qg):
                # one wo_group (2 q-chunks) as 5 chunks:
                # [transposes+evict], then per (qc, n-half) matmul+evict;
                # the last chunk DMAs the assembled [128, 2, D] block out.
                state = {}

                def c_transpose():
                    xt_ps = pp_tile([128, 2, 2, 128], bf16)
                    first = True
                    for j2 in range(2):
                        qc = qg * 2 + j2
                        for g2 in range(2):
                            nc.tensor.matmul(
                                xt_ps[:, g2, j2, :],
                                lhsT=xn_sb[:, qc, 2 * g2:2 * g2 + 2, :],
                                rhs=ident[:],
                                is_transpose=True,
                                start=first, stop=True,
                            )
                            first = False
                    nc.vector.tensor_copy(
                        xTn_sb[:, :, qg * 256:(qg + 1) * 256],
                        xt_ps[:].rearrange("p a b q -> p a (b q)"),
                    )

                def c_pso(j2, n2):
                    if j2 == 0 and n2 == 0:
                        state["ost"] = op_.tile([128, 2, D], f32, tag="ost",
                                                name="ost")
                    qc = qg * 2 + j2
                    pso = pp_tile([128, 512], f32)
                    for g2 in range(2):
                        nc.tensor.matmul(
                            pso[:],
                            lhsT=xTn_sb[:, g2, qc * 128:(qc + 1) * 128],
                            rhs=wo_sb[:, g2, n2 * 512:(n2 + 1) * 512],
                            start=(g2 == 0), stop=(g2 == 1),
                        )
                    nc.vector.tensor_copy(
                        state["ost"][:, j2, n2 * 512:(n2 + 1) * 512], pso[:])
                    if j2 == 1 and n2 == 1:
                        nc.sync.dma_start(
                            out=out_r[:, qg * 2:(qg + 1) * 2, :],
                            in_=state["ost"][:])

                return [c_transpose,
                        lambda: c_pso(0, 0), lambda: c_pso(0, 1),
                        lambda: c_pso(1, 0), lambda: c_pso(1, 1)]

            def wo_tail(qg):
                # tail-only: score rotation is idle -> [128,1024] pso gens,
                # per-q-chunk transpose/evict/DMA so the chain pipelines
                for j2 in range(2):
                    qc = qg * 2 + j2
                    xt_ps = pp_tile([128, 2, 128], bf16)
                    for g2 in range(2):
                        nc.tensor.matmul(
                            xt_ps[:, g2, :],
                            lhsT=xn_sb[:, qc, 2 * g2:2 * g2 + 2, :],
                            rhs=ident[:],
                            is_transpose=True,
                            start=(g2 == 0), stop=True,
                        )
                    nc.vector.tensor_copy(
                        xTn_sb[:, :, qc * 128:(qc + 1) * 128],
                        xt_ps[:])
                    ost = op_.tile([128, 1, D], f32, tag="ost", name="ost")
                    pso = st_tile()
                    for n2 in range(D // 512):
                        for g2 in range(2):
                            nc.tensor.matmul(
                                pso[:, n2 * 512:(n2 + 1) * 512],
                                lhsT=xTn_sb[:, g2, qc * 128:(qc + 1) * 128],
                                rhs=wo_sb[:, g2, n2 * 512:(n2 + 1) * 512],
                                start=(g2 == 0), stop=(g2 == 1),
                            )
                    # alternate evict engine: ScalarE is idle after the last
                    # exp, halving the DVE-gated tail cadence
                    if j2 == 0:
                        nc.scalar.copy(ost[:, 0, :], pso[:])
                    else:
                        nc.vector.tensor_copy(ost[:, 0, :], pso[:])
                    nc.sync.dma_start(out=out_r[:, qc:qc + 1, :], in_=ost[:])

            # ---- attention pipeline ----

            def attn_head(h, qb, fillers):
                # fillers: list of per-slot lists of callables (len 8)
                hb = (h % 2) * 64
                m = h // 2
                qsl = slice(qb * QB, (qb + 1) * QB)
                x_ps = psP.tile([128, 4, HD + 1], f32, tag="px", bufs=PX_BUFS,
                                name="xps")
                pe_ts = [None] * 8

                def pv_batch(pr):
                    pe_t = pe_ts[pr]
                    for half, kc in ((0, 2 * pr), (1, 2 * pr + 1)):
                        for qs in range(4):
                            nc.tensor.matmul(
                                x_ps[:, qs, :],
                                lhsT=pe_t[:, half * 512 + qs * 128:
                                          half * 512 + (qs + 1) * 128],
                                rhs=Vs_sb[:, kc, h, :],
                                start=(pr == 0 and half == 0 and qs == 0),
                                stop=(pr == SC // 2 - 1 and half == 1),
                            )

                def st_exp(pr):
                    kc0, kc1 = 2 * pr, 2 * pr + 1
                    st_pair = st_tile()
                    nc.tensor.matmul(
                        st_pair[:, 0:512],
                        lhsT=KTt[hb:hb + 64, m, kc0 * 128:(kc0 + 1) * 128],
                        rhs=QT[hb:hb + 64, m, qsl],
                        start=True, stop=True,
                    )
                    nc.tensor.matmul(
                        st_pair[:, 512:1024],
                        lhsT=KTt[hb:hb + 64, m, kc1 * 128:(kc1 + 1) * 128],
                        rhs=QT[hb:hb + 64, m, qsl],
                        start=True, stop=True,
                    )
                    pe_t = sp.tile([128, 1024], bf16, tag="stexp", name="pet")
                    nc.scalar.activation(pe_t[:], st_pair[:], EXP, scale=0.125)
                    pe_ts[pr] = pe_t

                # scores run two slots ahead of their exp's slot so the
                # psum-WAR chain (bank freed by exp(i-3)) never paces the
                # exp stream; PV runs four slots behind, with the last four
                # batches + norm deferred into the NEXT block's fillers so
                # DMA-gated V work and the px-WAR handoff never sit in front
                # of the next block's score matmuls in PE's in-order queue.
                st_exp(0)
                st_exp(1)
                for pr in range(8):
                    if pr + 2 < 8:
                        st_exp(pr + 2)
                    if pr >= 5:
                        # lag 5: every PV batch sits after ALL eight score
                        # issues (last is st(7) at pr=5), so a late px
                        # handoff can never stall the exp stream
                        pv_batch(pr - 5)
                    for f in fillers[pr]:
                        f()

                def norm_fn():
                    rc = npl.tile([128, 4], f32, tag="rc", name="rc")
                    nc.vector.reciprocal(rc[:], x_ps[:, :, HD:HD + 1])
                    for qs in range(4):
                        nc.vector.tensor_scalar_mul(
                            xn_sb[:, qb * 4 + qs, h, :],
                            x_ps[:, qs, 0:HD],
                            rc[:, qs:qs + 1],
                        )

                return [lambda pr=pr: pv_batch(pr) for pr in (3, 4, 5, 6)] + \
                    [lambda: (pv_batch(7), norm_fn())]

            def spread(chunks, nslots):
                # distribute chunks over nslots slots, front-loaded
                out = [[] for _ in range(nslots)]
                for i, c in enumerate(chunks):
                    out[i * nslots // len(chunks)].append(c)
                return out

            # ---- schedule: qb-major walk ----
            # pq[t][m][j]: 4-chunk list projecting seq-quarter j of (t, m);
            # vq[g]: 8-chunk list for V seq-group g (2 gens of 4)
            pq = [[[qk_proj_chunks_q(t, m, j) for j in range(4)]
                   for m in range(2)] for t in range(2)]
            vq = [v_proj_chunks(g) for g in range(4)]

            # upfront: quarter-0 Q/K and quarter-1 K for the m=0 pair
            # (sts are issued two slots early, so K quarters must land two
            # slots sooner than their exp's slot)
            for c in pq[0][0][0] + pq[1][0][0] + pq[1][0][1]:
                c()

            empty = [[] for _ in range(8)]
            fill = {}
            # walk: (qb0,qb1) x m0 heads, then their m1 heads, then
            # (qb2,qb3) likewise. m=1 projections leave window 0 entirely;
            # each qb finishes 2+ blocks before its wo chunks are scheduled.
            walk = [(0, 0), (0, 1), (1, 0), (1, 1),
                    (0, 2), (0, 3), (1, 2), (1, 3),
                    (2, 0), (2, 1), (3, 0), (3, 1),
                    (2, 2), (2, 3), (3, 2), (3, 3)]
            fill[(0, 0)] = [
                vq[0][0:4] + pq[1][0][2][0:2],   # Vg0 (xv0@16), Kq2 (18.9)
                vq[0][4:8] + pq[1][0][2][2:4],
                vq[1][0:4],                      # Vg1 gen0 (xv1@21.8)
                pq[1][0][3],                     # Kq3 (24.7, hard st(6) dl)
                vq[1][4:8],
                [], [], [],
            ]
            fill[(0, 1)] = [
                vq[2][0:4],                      # Vg2 (xv2@27.6): runs while
                vq[2][4:8] + pq[0][0][1][0:2],   # (0,1)'s own sts stream
                pq[0][0][1][2:4] + vq[3][0:4],   # Qm0q1 before Vg3 in the pp
                vq[3][4:8],                      # chain: it gates (1,0)
                [], [],
                pq[1][1][0][0:2],
                pq[1][1][0][2:4],
            ]
            fill[(1, 0)] = spread(pq[1][1][1] + pq[1][1][2], 8)
            fill[(1, 1)] = spread(pq[1][1][3] + pq[0][1][0] + pq[0][1][1], 8)
            fill[(0, 2)] = []
            fill[(0, 3)] = []
            fill[(1, 2)] = [pq[0][0][2][0:2], pq[0][0][2][2:4], [], [],
                            []] + spread(wo_chunks(0), 3)
            fill[(1, 3)] = spread(wo_chunks(1), 8)
            fill[(2, 0)] = [pq[0][0][3][0:2], pq[0][0][3][2:4], [], [],
                            []] + spread(wo_chunks(2), 3)
            fill[(2, 1)] = spread(wo_chunks(3), 8)
            fill[(3, 0)] = [pq[0][1][2][0:2], pq[0][1][2][2:4], [], []] + [
                [], [], [], []]
            fill[(3, 1)] = [pq[0][1][3][0:2], pq[0][1][3][2:4], [], []] + [
                [], [], [], []]
            fill[(2, 2)] = []
            fill[(2, 3)] = []
            fill[(3, 2)] = [[], [], [], [], []] + spread(wo_chunks(4), 3)
            fill[(3, 3)] = spread(wo_chunks(5), 5) + [[], [], []]
            fins = []
            for qb, h in walk:
                ch = fill[(qb, h)]
                if not ch:
                    ch = [[] for _ in range(8)]
                elif not isinstance(ch[0], list):
                    ch = spread(ch, 8)
                ch = [list(s) for s in ch]
                # append: same-slot fillers (e.g. V gens) must precede
                # the finisher PV batches that read them
                for i, f in enumerate(fins):
                    ch[i].append(f)
                fins = attn_head(h, qb, ch)
            for f in fins:
                f()
            wo_tail(6)
            wo_tail(7)

    import concourse.mybir as mybir_mod
    _split_excess_waits(nc, mybir_mod)
    return nc


def kernel(q, k, v, mask, Wq, bq, Wk, bk, Wv, bv, Wo, bo):
    global _cached_nc, _last_result
    from concourse.bass_utils import run_bass_kernel_spmd

    if _cached_nc is None:
        _cached_nc = _build()
    nc = _cached_nc

    bf = ml_dtypes.bfloat16
    q = np.asarray(q); k = np.asarray(k); v = np.asarray(v)
    Wq = np.asarray(Wq); Wk = np.asarray(Wk); Wv = np.asarray(Wv)
    Wo = np.asarray(Wo)

    xt = {}
    for b in range(B):
        xt[("q", b)] = np.ascontiguousarray(q[b].T).astype(bf)
        xt[("k", b)] = np.ascontiguousarray(k[b].T).astype(bf)
        xt[("v", b)] = np.ascontiguousarray(v[b].T).astype(bf)

    in_maps = []
    for c in range(NCORES):
        b, hg = c // GROUPS, c % GROUPS
        sl = slice(hg * DSL, (hg + 1) * DSL)
        wqkv = np.ascontiguousarray(
            np.concatenate([Wq[:, sl], Wk[:, sl], Wv[:, sl]], axis=1)
        ).astype(bf)
        wo = np.ascontiguousarray(Wo[sl, :]).astype(bf)
        in_maps.append({
            "xtq": xt[("q", b)], "xtk": xt[("k", b)], "xtv": xt[("v", b)],
            "wqkv": wqkv, "wo": wo,
        })

    try:
        res = run_bass_kernel_spmd(nc, in_maps, list(range(NCORES)),
                                   trace=TRACE, **TRACE_KW)
    except ModuleNotFoundError:
        # no NTFF profiling hook in this axon client; run without trace
        res = run_bass_kernel_spmd(nc, in_maps, list(range(NCORES)))
    _last_result = res

    out = np.empty((B, S, D), np.float32)
    for b in range(B):
        acc = res.results[GROUPS * b]["out"].copy()
        for j in range(1, GROUPS):
            acc += res.results[GROUPS * b + j]["out"]
        out[b] = acc
    return out
